# revision 1
# baseline (speedup 1.0000x reference)
"""Multi-head attention forward (B=2, N=2048, C=1024, H=16) on 8 TRN2 NeuronCores.

Tensor-parallel over heads: core c owns heads {2c, 2c+1}. Each core computes
QKV projection for its heads, full attention for its 4 (batch, head)
instances, and a partial output projection against its 128 rows of w_proj.
The host sums the 8 partial projections and adds the bias (row-parallel TP;
the all-reduce is the host-side unshard).

Per-core layouts (all matmul inputs bf16, PSUM accumulation f32):
  xT    [1024, 4096]  x^T, channel-major (replicated)
  wqk   [1024, 256]   [Wq_h0|Wq_h1|Wk_h0|Wk_h1] columns, Wq pre-scaled by D^-0.5
  wv    [1024, 128]   [Wv_h0|Wv_h1]
  wproj [128, 1024]   rows 128c:128c+128 of w_proj
  out   [4096, 1024]  bf16 partial projection output

Attention processes BOTH heads per (batch, q-block): the two S^T matmuls
(K=64 contraction each) are issued back-to-back with tile_position row
tiles (h0 -> array rows 0:63, h1 -> rows 64:127, auto-derived from the
qk_sb base partitions) so they run CONCURRENTLY in disjoint halves of the
PE array -- 2x throughput on the half-contraction score matmuls. Per
k-tile, one [128, 1024] PSUM tile holds [S^T_h0 | S^T_h1] in two banks and
a single ACT exp covers both heads. O^T accumulation per head:
po[*,q] += [V|1|0pad].T @ exp(S^T) with the 128-wide padded V stationary;
row 64 of po accumulates the softmax denominator l for free. Normalize via
fast-reciprocal + gpsimd partition-broadcast + one DVE multiply; head-1
rows repacked to partitions 64:128 by an SBUF->SBUF DMA so the projection
is a single K=128 GEMM.

Scheduling: all independent PE work (stage 1, projections) is woven into
the attention loops' ACT-wait slots so the PE instruction stream never
idles (HAM stays at full clock). Exp->O matmuls are skewed 3 kp
iterations; each q-block's last 6 O-accumulations plus its normalize
chain are CARRIED into the next q-block's kp0-2 slots so no boundary
drain stalls the exp stream. DMA descriptor issue (~0.7us each per
sequencer) is kept off the scalar/vector queues except for the 16
first-matmul-critical chunks, which are spread across sync/scalar/gpsimd
and finish before those engines' first real work.
"""

import numpy as np
import ml_dtypes

import concourse.bass as bass
import concourse.tile as tile
from concourse import bacc, mybir
from concourse.bass_utils import run_bass_kernel_spmd
from concourse.masks import make_identity

B, N, C = 2, 2048, 1024
H = 16
D = C // H          # 64
SCALE = D ** -0.5
NCORES = 8
T = B * N           # 4096 tokens
KT = C // 128       # 8 k-tiles over the C contraction
TOK_TILES = T // 128  # 32
NK = N // 128       # 16 key tiles per sequence
QB = 512            # q block width
NQB = N // QB       # 4
BF = mybir.dt.bfloat16
F32 = mybir.dt.float32

_NC_CACHE = {}


def build():
    nc = bacc.Bacc("TRN2", target_bir_lowering=False, debug=False,
                   num_devices=NCORES)
    xT = nc.dram_tensor("xT", [C, T], BF, kind="ExternalInput").ap()
    wqk = nc.dram_tensor("wqk", [C, 256], BF, kind="ExternalInput").ap()
    wv = nc.dram_tensor("wv", [C, 128], BF, kind="ExternalInput").ap()
    wproj = nc.dram_tensor("wproj", [128, C], BF, kind="ExternalInput").ap()
    out = nc.dram_tensor("out", [T, C], BF, kind="ExternalOutput").ap()

    with tile.TileContext(nc) as tc:
        with tc.tile_pool(name="const", bufs=1) as const, \
             tc.tile_pool(name="work", bufs=2) as work, \
             tc.tile_pool(name="ps", bufs=2, space="PSUM") as ps:

            xt_sb = const.tile([128, KT, T], BF, tag="xt")
            wqk_sb = const.tile([128, KT, 256], BF, tag="wqk")
            wv_sb = const.tile([128, KT, 128], BF, tag="wv")
            wproj_sb = const.tile([128, C], BF, tag="wproj")
            qk_sb = const.tile([128, 2, T], BF, tag="qk")   # [qchan|kchan, token]
            vt_sb = const.tile([128, T], BF, tag="vt")      # V^T [vchan, token]
            v_sb = const.tile([128, TOK_TILES, 2, 128], BF, tag="v")  # per head [V|1|0pad]
            # normalized O^T [dchan, token]; h0 rows 0:64 written by DVE,
            # h1 rows 64:128 filled by SBUF->SBUF DMA repack from ot1_sb
            otp_sb = const.tile([128, T], BF, tag="otp")
            ot1_sb = const.tile([64, T], BF, tag="ot1")
            ident = const.tile([128, 128], BF, tag="ident")

            # DMA descriptor issue costs ~0.7us per descriptor per queue
            # sequencer. The 16 first-S-pair-critical chunks (8 wqk + 8 x^T
            # nt0) are spread across ALL FOUR queues -- the scalar/vector
            # queues each take 4, finishing before those engines' first
            # real work (exps / PSUM copies) is reachable anyway. Everything
            # later goes on sync/gpsimd only so it can never delay an exp
            # or a PSUM-evacuation copy behind it in the strict-FIFO queue.
            def xt_dma(eng, nt, kt):
                eng.dma_start(
                    out=xt_sb[:, kt, nt * 512:(nt + 1) * 512],
                    in_=xT[kt * 128:(kt + 1) * 128,
                           nt * 512:(nt + 1) * 512])

            for kt in range(KT):
                eng = nc.sync if kt % 2 == 0 else nc.scalar
                eng.dma_start(out=wqk_sb[:, kt, :],
                              in_=wqk[kt * 128:(kt + 1) * 128, :])
                if kt % 2 == 0:
                    xt_dma(nc.gpsimd, 0, kt)
                else:
                    xt_dma(nc.sync if kt % 4 == 1 else nc.gpsimd, 0, kt)

            make_identity(nc, ident[:])
            nc.vector.memset(v_sb[:, :, :, 64:65], 1.0)

            for kt in range(KT):
                xt_dma(nc.sync if kt % 2 == 0 else nc.gpsimd, 1, kt)
                nc.gpsimd.dma_start(out=wv_sb[:, kt, :],
                                    in_=wv[kt * 128:(kt + 1) * 128, :])
            nc.gpsimd.dma_start(out=wproj_sb[:], in_=wproj[:, :])
            for nt in range(2, T // 512):
                for kt in range(KT):
                    xt_dma(nc.sync if kt % 2 == 0 else nc.gpsimd, nt, kt)

            # -- work units -------------------------------------------------
            def emit_qk(mt, nt):
                pmm = ps.tile([128, 512], F32, tag="mm")
                for kt in range(KT):
                    nc.tensor.matmul(
                        pmm[:],
                        wqk_sb[:, kt, mt * 128:(mt + 1) * 128],
                        xt_sb[:, kt, nt * 512:(nt + 1) * 512],
                        start=(kt == 0), stop=(kt == KT - 1))
                nc.vector.tensor_copy(
                    qk_sb[:, mt, nt * 512:(nt + 1) * 512], pmm[:])

            def emit_vt(nt):
                # V^T chunk [128 vchan, 512 tok] with wv stationary
                pv = ps.tile([128, 512], F32, tag="mm")
                for kt in range(KT):
                    nc.tensor.matmul(
                        pv[:],
                        wv_sb[:, kt, :],
                        xt_sb[:, kt, nt * 512:(nt + 1) * 512],
                        start=(kt == 0), stop=(kt == KT - 1))
                nc.vector.tensor_copy(
                    vt_sb[:, nt * 512:(nt + 1) * 512], pv[:])

            def emit_qk_split(mt, nt):
                # 8-chunk GEMM split into two 4-chunk halves emitted in
                # CONSECUTIVE extras slots (accumulation group + psum tile
                # stay open across the pair; at most one other mm-tag
                # allocation may occur between halves -- placement ensures
                # zero)
                st = {}

                def a():
                    pmm = ps.tile([128, 512], F32, tag="mm", name="pmm")
                    st["p"] = pmm
                    for kt in range(KT // 2):
                        nc.tensor.matmul(
                            pmm[:],
                            wqk_sb[:, kt, mt * 128:(mt + 1) * 128],
                            xt_sb[:, kt, nt * 512:(nt + 1) * 512],
                            start=(kt == 0), stop=False)

                def b():
                    pmm = st["p"]
                    for kt in range(KT // 2, KT):
                        nc.tensor.matmul(
                            pmm[:],
                            wqk_sb[:, kt, mt * 128:(mt + 1) * 128],
                            xt_sb[:, kt, nt * 512:(nt + 1) * 512],
                            start=False, stop=(kt == KT - 1))
                    nc.vector.tensor_copy(
                        qk_sb[:, mt, nt * 512:(nt + 1) * 512], pmm[:])

                return a, b

            def emit_vt_split(nt):
                st = {}

                def a():
                    pv = ps.tile([128, 512], F32, tag="mm", name="pv")
                    st["p"] = pv
                    for kt in range(KT // 2):
                        nc.tensor.matmul(
                            pv[:],
                            wv_sb[:, kt, :],
                            xt_sb[:, kt, nt * 512:(nt + 1) * 512],
                            start=(kt == 0), stop=False)

                def b():
                    pv = st["p"]
                    for kt in range(KT // 2, KT):
                        nc.tensor.matmul(
                            pv[:],
                            wv_sb[:, kt, :],
                            xt_sb[:, kt, nt * 512:(nt + 1) * 512],
                            start=False, stop=(kt == KT - 1))
                    nc.vector.tensor_copy(
                        vt_sb[:, nt * 512:(nt + 1) * 512], pv[:])

                return a, b

            def emit_vtr(t):
                # PE-transpose V^T tile back to token-major [128 tok, 128 d]
                pt = ps.tile([128, 128], BF, tag="mm")
                nc.tensor.transpose(
                    pt[:], vt_sb[:, t * 128:(t + 1) * 128], ident[:])
                nc.vector.tensor_copy(v_sb[:, t, 0, 0:64], pt[:, 0:64])
                nc.vector.tensor_copy(v_sb[:, t, 1, 0:64], pt[:, 64:128])
                # pad cols feed only po rows 65:127 (never read)
                nc.vector.memset(v_sb[:, t, :, 65:128], 0.0)

            def emit_proj(g, use_scalar=False, dma_eng=None):
                # out_tile = O^T_packed.T @ wproj (K=128, both heads)
                ob = work.tile([128, C], BF, tag="outstage", bufs=4)
                for ntile in range(2):
                    pmm = ps.tile([128, 512], F32, tag="mm")
                    nc.tensor.matmul(
                        pmm[:],
                        otp_sb[:, g * 128:(g + 1) * 128],
                        wproj_sb[:, ntile * 512:(ntile + 1) * 512],
                        start=True, stop=True)
                    if use_scalar and ntile == 1:
                        nc.scalar.copy(ob[:, 512:1024], pmm[:])
                    else:
                        nc.vector.tensor_copy(
                            ob[:, ntile * 512:(ntile + 1) * 512], pmm[:])
                (dma_eng or nc.sync).dma_start(
                    out=out[g * 128:(g + 1) * 128, :], in_=ob[:])

            # One attention q-block, BOTH heads. Per k-tile: two
            # concurrently-row-tiled S^T matmuls into [S_h0|S_h1] PSUM
            # halves, one exp over both, two O^T accumulations (skewed 3
            # kp iterations). extras[kp] are unrelated PE work units woven
            # into the ACT-wait slots so the PE never idles. The last 6
            # O accumulations + the normalize chain are CARRIED into the
            # next q-block's kp0-2 slots so the next block's S-pairs reach
            # ACT without a boundary drain stall; po slots (bufs=1) free at
            # the carry-normalize (kp2), just before this block's first own
            # O pop (kp3).
            def emit_s2_pair(b, qb, extras=None, carry=None, last=False):
                q0 = b * N + qb * QB
                po0 = ps.tile([128, 512], F32, tag="o0", bufs=1)
                po1 = ps.tile([128, 512], F32, tag="o1", bufs=1)
                from collections import deque as _dq
                pending = _dq()

                def pop_o():
                    es, kt = pending.popleft()
                    vt = b * NK + kt
                    nc.tensor.matmul(
                        po0[:], v_sb[:, vt, 0, :], es[:, 0:512],
                        start=(kt == 0), stop=(kt == NK - 1))
                    nc.tensor.matmul(
                        po1[:], v_sb[:, vt, 1, :], es[:, 512:1024],
                        start=(kt == 0), stop=(kt == NK - 1))

                def normalize(heads=(0, 1)):
                    # normalize O^T rows by 1/l per head: fast reciprocal
                    # of row 64, partition-broadcast, one DVE multiply
                    for h, po in (((0, po0), (1, po1))[hh] for hh in heads):
                        lrow = work.tile([1, 512], F32, tag="lrow")
                        nc.vector.tensor_copy(lrow[:], po[64:65, :])
                        linv = work.tile([1, 512], F32, tag="linv")
                        nc.vector.reciprocal_approx_fast(linv[:], lrow[:])
                        lb = work.tile([64, 512], F32, tag="lb")
                        nc.gpsimd.partition_broadcast(lb[:], linv[:])
                        if h == 0:
                            nc.vector.tensor_mul(
                                otp_sb[0:64, q0:q0 + QB], po[0:64, :], lb[:])
                        else:
                            nc.vector.tensor_mul(
                                ot1_sb[:, q0:q0 + QB], po[0:64, :], lb[:])
                            # repack h1 rows into partitions 64:128 of the
                            # packed O^T tile (cross-partition SBUF->SBUF
                            # DMA on the gpsimd queue)
                            nc.gpsimd.dma_start(
                                out=otp_sb[64:128, q0:q0 + QB],
                                in_=ot1_sb[:, q0:q0 + QB])

                def s_exp(kp, j):
                    kt = kp * 2 + j
                    k0 = b * N + kt * 128
                    pst = ps.tile([128, 1024], F32, tag="s")
                    nc.tensor.matmul(
                        pst[:, 0:512],
                        qk_sb[0:64, 1, k0:k0 + 128],
                        qk_sb[0:64, 0, q0:q0 + QB],
                        start=True, stop=True)
                    nc.tensor.matmul(
                        pst[:, 512:1024],
                        qk_sb[64:128, 1, k0:k0 + 128],
                        qk_sb[64:128, 0, q0:q0 + QB],
                        start=True, stop=True)
                    es = work.tile([128, 1024], BF, tag="es", bufs=13)
                    nc.scalar.activation(
                        es[:], pst[:], mybir.ActivationFunctionType.Exp)
                    pending.append((es, kt))

                for kp in range(NK // 2):
                    # O-pairs + extras run BEFORE this iteration's S-pairs
                    # so the S matmuls' psum-slot waits (on exp reads) have
                    # cleared by the time the engine reaches them
                    if carry is not None:
                        carry[0]()
                        carry[0]()
                        if carry[1]():
                            carry = None
                    else:
                        if len(pending) >= 6:
                            pop_o()
                            pop_o()
                    if extras:
                        for u in extras.get(kp, ()):
                            u()
                    s_exp(kp, 0)
                    s_exp(kp, 1)
                if last:
                    # drain head-by-head: h0's normalize chain (DVE+gpsimd)
                    # overlaps h1's remaining O matmuls on the PE
                    rem = list(pending)
                    pending.clear()
                    for es, kt in rem:
                        nc.tensor.matmul(
                            po0[:], v_sb[:, b * NK + kt, 0, :], es[:, 0:512],
                            start=(kt == 0), stop=(kt == NK - 1))
                    normalize(heads=(0,))
                    for es, kt in rem:
                        nc.tensor.matmul(
                            po1[:], v_sb[:, b * NK + kt, 1, :],
                            es[:, 512:1024],
                            start=(kt == 0), stop=(kt == NK - 1))
                    normalize(heads=(1,))
                    return None

                def carry_pop():
                    if pending:
                        pop_o()

                def carry_fin():
                    if pending:
                        return False
                    normalize()
                    return True

                return (carry_pop, carry_fin)

            # -- schedule ---------------------------------------------------
            def U(f, *a):
                return lambda: f(*a)

            # minimal prefix for (b0, qb0): the K and Q chunks the first
            # S-pairs need, K/Q interleaved per k-chunk so both finish one
            # matmul after the last DMA chunk lands
            pK = ps.tile([128, 512], F32, tag="mm")
            pQ = ps.tile([128, 512], F32, tag="mm")
            for kt in range(KT):
                nc.tensor.matmul(pK[:], wqk_sb[:, kt, 128:256],
                                 xt_sb[:, kt, 0:512],
                                 start=(kt == 0), stop=(kt == KT - 1))
                nc.tensor.matmul(pQ[:], wqk_sb[:, kt, 0:128],
                                 xt_sb[:, kt, 0:512],
                                 start=(kt == 0), stop=(kt == KT - 1))
            nc.vector.tensor_copy(qk_sb[:, 1, 0:512], pK[:])
            nc.vector.tensor_copy(qk_sb[:, 0, 0:512], pQ[:])
            # remaining batch-0 stage-1 woven into (b0,qb0) just ahead of
            # consumers (S kp needs K chunk kp//2; own O pops start at kp3
            # consuming k-tiles 2kp-6,2kp-5; carry-in drains at kp0-2)
            cy = emit_s2_pair(0, 0, extras={
                0: [U(emit_vt, 0)],
                1: [U(emit_qk, 1, 1), U(emit_vtr, 0), U(emit_vtr, 1)],
                2: [U(emit_vt, 1), U(emit_vtr, 2), U(emit_vtr, 3)],
                3: [U(emit_qk, 1, 2), U(emit_vtr, 4), U(emit_vtr, 5)],
                4: [U(emit_vt, 2), U(emit_vtr, 6), U(emit_vtr, 7)],
                5: [U(emit_qk, 1, 3), U(emit_vtr, 8), U(emit_vtr, 9)],
                6: [U(emit_vt, 3), U(emit_qk, 0, 1)],
                7: [U(emit_vtr, 10), U(emit_vtr, 11), U(emit_vtr, 12),
                    U(emit_vtr, 13)],
            })
            # batch-1 stage-1 spread across batch 0's remaining q-blocks and
            # (b1,qb0) itself, with the 8-chunk GEMM units SPLIT into
            # 4-chunk halves in consecutive extras slots (halves share an
            # open accumulation group; placement guarantees no intervening
            # mm-tag allocation). Batch-0 projections start as soon as both
            # heads of a q-block are normalized+repacked.
            qk14 = emit_qk_split(1, 4)
            qk04 = emit_qk_split(0, 4)
            vt4s = emit_vt_split(4)
            qk02 = emit_qk_split(0, 2)
            cy = emit_s2_pair(0, 1, carry=cy, extras={
                0: [U(emit_vtr, 14), U(emit_vtr, 15)],
                1: [qk14[0]],
                2: [qk14[1], qk04[0]],
                3: [qk04[1]],
                4: [vt4s[0]],
                5: [vt4s[1]],
                6: [qk02[0]],
                7: [qk02[1], U(emit_vtr, 16), U(emit_vtr, 17)],
            })
            qk15 = emit_qk_split(1, 5)
            qk05 = emit_qk_split(0, 5)
            qk03 = emit_qk_split(0, 3)
            vt5s = emit_vt_split(5)
            cy = emit_s2_pair(0, 2, carry=cy, extras={
                0: [U(emit_vtr, 18), U(emit_vtr, 19)],
                1: [qk15[0]],
                2: [qk15[1]],
                3: [qk05[0]],
                4: [qk05[1]],
                5: [qk03[0]],
                6: [qk03[1]],
                7: [vt5s[0]],
            })
            qk16 = emit_qk_split(1, 6)
            qk06 = emit_qk_split(0, 6)
            vt6s = emit_vt_split(6)
            cy = emit_s2_pair(0, 3, carry=cy, extras={
                0: [vt5s[1], U(emit_vtr, 20), U(emit_vtr, 21)],
                1: [qk16[0]],
                2: [qk16[1], U(emit_vtr, 22), U(emit_vtr, 23)],
                3: [qk06[0]],
                4: [qk06[1]],
                5: [vt6s[0]],
                6: [vt6s[1], U(emit_proj, 0)],
                7: [U(emit_proj, 1)],
            })
            qk17 = emit_qk_split(1, 7)
            qk07 = emit_qk_split(0, 7)
            vt7s = emit_vt_split(7)
            cy = emit_s2_pair(1, 0, carry=cy, extras={
                0: [U(emit_vtr, 24), U(emit_vtr, 25)],
                1: [qk17[0]],
                2: [qk17[1], qk07[0]],
                3: [qk07[1], vt7s[0]],
                4: [vt7s[1], U(emit_vtr, 26), U(emit_vtr, 27)],
                5: [U(emit_vtr, 28), U(emit_vtr, 29)],
                6: [U(emit_vtr, 30), U(emit_vtr, 31)],
                7: [U(emit_proj, 2), U(emit_proj, 3)],
            })
            cy = emit_s2_pair(1, 1, carry=cy, extras={
                0: [U(emit_proj, 4), U(emit_proj, 5)],
                1: [U(emit_proj, 6), U(emit_proj, 7)],
                2: [U(emit_proj, 8)],
                4: [U(emit_proj, 9)],
                5: [U(emit_proj, 10)],
                6: [U(emit_proj, 11)],
                7: [U(emit_proj, 16)],
            })
            cy = emit_s2_pair(1, 2, carry=cy, extras={
                0: [U(emit_proj, 12), U(emit_proj, 13)],
                1: [U(emit_proj, 14), U(emit_proj, 15)],
                2: [U(emit_proj, 17)],
                4: [U(emit_proj, 18)],
                5: [U(emit_proj, 19)],
                7: [U(emit_proj, 20)],
            })
            emit_s2_pair(1, 3, carry=cy, last=True, extras={
                0: [U(emit_proj, 21), U(emit_proj, 22)],
                1: [U(emit_proj, 23)],
                3: [U(emit_proj, 24)],
                4: [U(emit_proj, 25)],
                5: [U(emit_proj, 26)],
                6: [U(emit_proj, 27)],
            })
            for i, g in enumerate(range(28, 32)):
                emit_proj(g, use_scalar=(i % 2 == 0),
                          dma_eng=nc.scalar if i % 2 == 1 else nc.sync)
    nc.compile()
    return nc


def make_in_maps(x, w_qkv, w_proj):
    bf = ml_dtypes.bfloat16
    x2 = x.reshape(T, C)
    xT_np = np.ascontiguousarray(x2.T).astype(bf)
    in_maps = []
    for c in range(NCORES):
        s = c * 128
        wq = w_qkv[:, s:s + 128] * SCALE
        wk = w_qkv[:, C + s:C + s + 128]
        wqk_np = np.ascontiguousarray(
            np.concatenate([wq, wk], axis=1)).astype(bf)
        wv_np = np.ascontiguousarray(
            w_qkv[:, 2 * C + s:2 * C + s + 128]).astype(bf)
        wproj_np = np.ascontiguousarray(w_proj[s:s + 128, :]).astype(bf)
        in_maps.append({"xT": xT_np, "wqk": wqk_np, "wv": wv_np,
                        "wproj": wproj_np})
    return in_maps


def kernel(x, w_qkv, w_proj, b_proj):
    x = np.asarray(x, dtype=np.float32)
    w_qkv = np.asarray(w_qkv, dtype=np.float32)
    w_proj = np.asarray(w_proj, dtype=np.float32)
    b_proj = np.asarray(b_proj, dtype=np.float32)

    if "nc" not in _NC_CACHE:
        _NC_CACHE["nc"] = build()
    nc = _NC_CACHE["nc"]

    in_maps = make_in_maps(x, w_qkv, w_proj)
    res = run_bass_kernel_spmd(nc, in_maps, list(range(NCORES)))
    acc = np.zeros((T, C), dtype=np.float32)
    for r in res.results:
        acc += np.asarray(r["out"], dtype=np.float32)
    acc += b_proj[None, :]
    return acc.reshape(B, N, C)



# revision 8
# speedup vs baseline: 1.1732x; 1.1732x over previous
"""Multi-head attention forward (B=2, N=2048, C=1024, H=16) on 8 TRN2 NeuronCores.

Tensor-parallel over heads: core c owns heads {2c, 2c+1}. Each core computes
QKV projection for its heads, full attention for its 4 (batch, head)
instances, and a partial output projection against its 128 rows of w_proj.
The host sums the 8 partial projections and adds the bias (row-parallel TP;
the all-reduce is the host-side unshard).

Per-core layouts (all matmul inputs bf16, PSUM accumulation f32):
  xT    [1024, 4096]  x^T, channel-major (replicated)
  wqk   [1024, 256]   [Wq_h0|Wq_h1|Wk_h0|Wk_h1] columns, Wq pre-scaled by D^-0.5
  wv    [1024, 128]   [Wv_h0|Wv_h1]
  wproj [128, 1024]   rows 128c:128c+128 of w_proj
  out   [4096, 1024]  bf16 partial projection output

Attention processes BOTH heads per (batch, q-block): the two S^T matmuls
(K=64 contraction each) are issued back-to-back with tile_position row
tiles so they run CONCURRENTLY in disjoint halves of the PE array. Per
k-tile, one [128, 1024] PSUM tile holds [S^T_h0 | S^T_h1] and a single ACT
exp covers both heads.

V is computed directly in token-major layout (stationary = x^T chunks,
moving = wv, N=128) -- no PE transposes. Slot layouts:
  slot0 (h0): [V_h0 (64) | 1 | zeros(63)]   -> po0 = [O_h0 0:64; l_h0 @64]
  slot1 (h1): [1 | zeros(63) | V_h1 (64)]   -> po1 = [l_h1 @0; O_h1 64:128]
so h1's normalized output writes otp partitions 64:128 DIRECTLY (the DVE
mul reads the broadcast tile at partitions 0:64; cross-offset in1 reads are
supported) -- no SBUF repack DMA. l rides the PV matmul via the ones column.

Startup: critical DMAs (wqk + x^T tokens 0:512) spread over all four
queues; ~40 dummy ident matmuls warm the PE HAM clock-gate during the DMA
wait; a short prefix (K tokens 0:256 at N=256, Q tokens 0:512) gets the
first S-pair and exp stream going ~12us earlier than a full-block prefix.

Scheduling: stage-1 and projection PE work is woven into the attention
loops' ACT-wait slots. Exp->O matmuls are skewed 3 kp iterations; each
q-block's last 6 O-accumulations drain 3-per-kp into the next block's
kp0/kp1 with the normalize chain issued at kp1 so the po PSUM banks free
before the block's own O pops begin at kp3.
"""

import numpy as np
import ml_dtypes

import concourse.bass as bass
import concourse.tile as tile
from concourse import bacc, mybir
from concourse.bass_utils import run_bass_kernel_spmd
from concourse.masks import make_identity

B, N, C = 2, 2048, 1024
H = 16
D = C // H          # 64
SCALE = D ** -0.5
NCORES = 8
T = B * N           # 4096 tokens
KT = C // 128       # 8 k-tiles over the C contraction
NK = N // 128       # 16 key tiles per sequence
QB = 512            # q block width
BF = mybir.dt.bfloat16
F32 = mybir.dt.float32

_NC_CACHE = {}


def build():
    nc = bacc.Bacc("TRN2", target_bir_lowering=False, debug=False,
                   num_devices=NCORES)
    xT = nc.dram_tensor("xT", [C, T], BF, kind="ExternalInput").ap()
    wqk = nc.dram_tensor("wqk", [C, 256], BF, kind="ExternalInput").ap()
    wv = nc.dram_tensor("wv", [C, 128], BF, kind="ExternalInput").ap()
    wproj = nc.dram_tensor("wproj", [128, C], BF, kind="ExternalInput").ap()
    out = nc.dram_tensor("out", [T, C], BF, kind="ExternalOutput").ap()

    with tile.TileContext(nc) as tc:
        with tc.tile_pool(name="const", bufs=1) as const, \
             tc.tile_pool(name="work", bufs=2) as work, \
             tc.tile_pool(name="ps", bufs=2, space="PSUM") as ps:

            xt_sb = const.tile([128, KT, T], BF, tag="xt")
            wqk_sb = const.tile([128, KT, 256], BF, tag="wqk")
            wv_sb = const.tile([128, KT, 128], BF, tag="wv")
            wproj_sb = const.tile([128, C], BF, tag="wproj")
            qk_sb = const.tile([128, 2, T], BF, tag="qk")   # [qchan|kchan, token]
            # v slots per 128-token tile: [h0: V|1|0pad, h1: 0pad|1|V]
            v_sb = const.tile([128, T // 128, 2, 128], BF, tag="v")
            otp_sb = const.tile([128, T], BF, tag="otp")    # normalized O^T packed
            ident = const.tile([128, 128], BF, tag="ident")

            # ---- critical DMAs: wqk (8 chunks) + x^T tokens 0:512 (8
            # chunks) spread as the FIRST 4 descriptors on each of the four
            # queues. Everything later goes on sync/gpsimd.
            def xt_dma(eng, nt, kt):
                eng.dma_start(
                    out=xt_sb[:, kt, nt * 512:(nt + 1) * 512],
                    in_=xT[kt * 128:(kt + 1) * 128,
                           nt * 512:(nt + 1) * 512])

            for kt in range(KT):
                (nc.sync if kt % 2 == 0 else nc.scalar).dma_start(
                    out=wqk_sb[:, kt, :], in_=wqk[kt * 128:(kt + 1) * 128, :])
                if kt % 2 == 0:
                    xt_dma(nc.gpsimd, 0, kt)
                else:
                    xt_dma(nc.sync if kt % 4 == 1 else nc.gpsimd, 0, kt)

            # ---- v-slot constants; then HAM-prewarm dummy matmuls (no DMA
            # deps) so the PE clock-gate opens during the DMA wait
            nc.vector.memset(ident[:], 1.0)
            nc.vector.memset(v_sb[:, :, 0, 64:65], 1.0)
            nc.vector.memset(v_sb[:, :, 0, 65:128], 0.0)
            nc.vector.memset(v_sb[:, :, 1, 0:1], 1.0)
            nc.vector.memset(v_sb[:, :, 1, 1:64], 0.0)
            pwarm = ps.tile([128, 512], F32, tag="mm")
            for i in range(40):
                nc.tensor.matmul(pwarm[:, 0:128], ident[:], ident[:],
                                 start=True, stop=True)

            # ---- remaining input DMAs on sync/gpsimd only
            for kt in range(KT):
                xt_dma(nc.sync if kt % 2 == 0 else nc.gpsimd, 1, kt)
                nc.gpsimd.dma_start(out=wv_sb[:, kt, :],
                                    in_=wv[kt * 128:(kt + 1) * 128, :])
            nc.gpsimd.dma_start(out=wproj_sb[:], in_=wproj[:, :])
            for nt in range(2, T // 512):
                for kt in range(KT):
                    xt_dma(nc.sync if kt % 2 == 0 else nc.gpsimd, nt, kt)

            # -- work units -------------------------------------------------
            def emit_qk(mt, nt):
                pmm = ps.tile([128, 512], F32, tag="mm")
                for kt in range(KT):
                    nc.tensor.matmul(
                        pmm[:],
                        wqk_sb[:, kt, mt * 128:(mt + 1) * 128],
                        xt_sb[:, kt, nt * 512:(nt + 1) * 512],
                        start=(kt == 0), stop=(kt == KT - 1))
                nc.vector.tensor_copy(
                    qk_sb[:, mt, nt * 512:(nt + 1) * 512], pmm[:])

            def emit_qk_split(mt, nt):
                # 8-chunk GEMM split into two 4-chunk halves emitted in
                # consecutive extras slots (accumulation group + psum tile
                # stay open across the pair)
                st = {}

                def a():
                    pmm = ps.tile([128, 512], F32, tag="mm", name="pmm")
                    st["p"] = pmm
                    for kt in range(KT // 2):
                        nc.tensor.matmul(
                            pmm[:],
                            wqk_sb[:, kt, mt * 128:(mt + 1) * 128],
                            xt_sb[:, kt, nt * 512:(nt + 1) * 512],
                            start=(kt == 0), stop=False)

                def b():
                    pmm = st["p"]
                    for kt in range(KT // 2, KT):
                        nc.tensor.matmul(
                            pmm[:],
                            wqk_sb[:, kt, mt * 128:(mt + 1) * 128],
                            xt_sb[:, kt, nt * 512:(nt + 1) * 512],
                            start=False, stop=(kt == KT - 1))
                    nc.vector.tensor_copy(
                        qk_sb[:, mt, nt * 512:(nt + 1) * 512], pmm[:])

                return a, b

            def emit_k256(j):
                # K chunk for tokens j*256:(j+1)*256 (k-tiles 2j, 2j+1);
                # one N=256 GEMM + one CAST, fine-grained for early blocks
                pmm = ps.tile([128, 512], F32, tag="mm")
                for kt in range(KT):
                    nc.tensor.matmul(
                        pmm[:, 0:256],
                        wqk_sb[:, kt, 128:256],
                        xt_sb[:, kt, j * 256:(j + 1) * 256],
                        start=(kt == 0), stop=(kt == KT - 1))
                nc.vector.tensor_copy(
                    qk_sb[:, 1, j * 256:(j + 1) * 256], pmm[:, 0:256])

            def emit_v(t):
                # token-major V for 128-token tile t, BOTH heads:
                # out[tok, vchan] = sum_k xT[k, tok-tile]^T @ wv[k, :]
                pv = ps.tile([128, 512], F32, tag="mm")
                for kt in range(KT):
                    nc.tensor.matmul(
                        pv[:, 0:128],
                        xt_sb[:, kt, t * 128:(t + 1) * 128],
                        wv_sb[:, kt, :],
                        start=(kt == 0), stop=(kt == KT - 1))
                nc.vector.tensor_copy(v_sb[:, t, 0, 0:64], pv[:, 0:64])
                nc.vector.tensor_copy(v_sb[:, t, 1, 64:128], pv[:, 64:128])

            def emit_proj(g, evac=None, dma_eng=None):
                # out_tile = O^T_packed.T @ wproj (K=128, both heads)
                ob = work.tile([128, C], BF, tag="outstage", bufs=4)
                for ntile in range(2):
                    pmm = ps.tile([128, 512], F32, tag="mm")
                    nc.tensor.matmul(
                        pmm[:],
                        otp_sb[:, g * 128:(g + 1) * 128],
                        wproj_sb[:, ntile * 512:(ntile + 1) * 512],
                        start=True, stop=True)
                    if evac == "scalar":
                        nc.scalar.copy(
                            ob[:, ntile * 512:(ntile + 1) * 512], pmm[:])
                    elif evac == "mixed" and ntile == 1:
                        nc.scalar.copy(ob[:, 512:1024], pmm[:])
                    else:
                        nc.vector.tensor_copy(
                            ob[:, ntile * 512:(ntile + 1) * 512], pmm[:])
                (dma_eng or nc.sync).dma_start(
                    out=out[g * 128:(g + 1) * 128, :], in_=ob[:])

            # One attention q-block, BOTH heads.
            def emit_s2_pair(b, qb, extras=None, post=None, carry=None,
                             last=False):
                q0 = b * N + qb * QB
                po0 = ps.tile([128, 512], F32, tag="o0", bufs=1)
                po1 = ps.tile([128, 512], F32, tag="o1", bufs=1)
                from collections import deque as _dq
                pending = _dq()

                def pop_o():
                    es, kt = pending.popleft()
                    vt = b * NK + kt
                    nc.tensor.matmul(
                        po0[:], v_sb[:, vt, 0, :], es[:, 0:512],
                        start=(kt == 0), stop=(kt == NK - 1))
                    nc.tensor.matmul(
                        po1[:], v_sb[:, vt, 1, :], es[:, 512:1024],
                        start=(kt == 0), stop=(kt == NK - 1))

                def normalize(heads=(0, 1)):
                    # h0: l at po0 row 64, O at rows 0:64 -> otp[0:64]
                    # h1: l at po1 row 0, O at rows 64:128 -> otp[64:128]
                    for h in heads:
                        po = po0 if h == 0 else po1
                        lrow = work.tile([1, 512], F32, tag="lrow")
                        nc.vector.tensor_copy(
                            lrow[:], po[64:65, :] if h == 0 else po[0:1, :])
                        nc.vector.reciprocal_approx_fast(lrow[:], lrow[:])
                        lb = work.tile([64, 512], F32, tag="lb")
                        nc.gpsimd.partition_broadcast(lb[:], lrow[:])
                        if h == 0:
                            nc.vector.tensor_mul(
                                otp_sb[0:64, q0:q0 + QB], po[0:64, :], lb[:])
                        else:
                            nc.vector.tensor_mul(
                                otp_sb[64:128, q0:q0 + QB], po[64:128, :],
                                lb[:])

                def s_exp(kp, j):
                    kt = kp * 2 + j
                    k0 = b * N + kt * 128
                    pst = ps.tile([128, 1024], F32, tag="s")
                    nc.tensor.matmul(
                        pst[:, 0:512],
                        qk_sb[0:64, 1, k0:k0 + 128],
                        qk_sb[0:64, 0, q0:q0 + QB],
                        start=True, stop=True)
                    nc.tensor.matmul(
                        pst[:, 512:1024],
                        qk_sb[64:128, 1, k0:k0 + 128],
                        qk_sb[64:128, 0, q0:q0 + QB],
                        start=True, stop=True)
                    es = work.tile([128, 1024], BF, tag="es", bufs=13)
                    nc.scalar.activation(
                        es[:], pst[:], mybir.ActivationFunctionType.Exp)
                    pending.append((es, kt))

                for kp in range(NK // 2):
                    if carry is not None:
                        # drain 3 pops at kp0, 3 + normalize at kp1
                        carry[0]()
                        carry[0]()
                        carry[0]()
                        if carry[1]():
                            carry = None
                    else:
                        if len(pending) >= 6:
                            pop_o()
                            pop_o()
                    if extras:
                        for u in extras.get(kp, ()):
                            u()
                    s_exp(kp, 0)
                    s_exp(kp, 1)
                    if post:
                        for u in post.get(kp, ()):
                            u()
                if last:
                    # drain head-by-head: h0's normalize chain overlaps h1's
                    # remaining O matmuls on the PE
                    rem = list(pending)
                    pending.clear()
                    for es, kt in rem:
                        nc.tensor.matmul(
                            po0[:], v_sb[:, b * NK + kt, 0, :], es[:, 0:512],
                            start=(kt == 0), stop=(kt == NK - 1))
                    normalize(heads=(0,))
                    for es, kt in rem:
                        nc.tensor.matmul(
                            po1[:], v_sb[:, b * NK + kt, 1, :],
                            es[:, 512:1024],
                            start=(kt == 0), stop=(kt == NK - 1))
                    normalize(heads=(1,))
                    return None

                def carry_pop():
                    if pending:
                        pop_o()

                def carry_fin():
                    if pending:
                        return False
                    normalize()
                    return True

                return (carry_pop, carry_fin)

            # -- schedule ---------------------------------------------------
            def U(f, *a):
                return lambda: f(*a)

            # minimal prefix for (b0, qb0): K tokens 0:256 (N=256) and the
            # full Q block 0:512, K/Q interleaved per k-chunk
            pK = ps.tile([128, 512], F32, tag="mm")
            pQ = ps.tile([128, 512], F32, tag="mm")
            for kt in range(KT):
                nc.tensor.matmul(pK[:, 0:256], wqk_sb[:, kt, 128:256],
                                 xt_sb[:, kt, 0:256],
                                 start=(kt == 0), stop=(kt == KT - 1))
                nc.tensor.matmul(pQ[:], wqk_sb[:, kt, 0:128],
                                 xt_sb[:, kt, 0:512],
                                 start=(kt == 0), stop=(kt == KT - 1))
            nc.vector.tensor_copy(qk_sb[:, 1, 0:256], pK[:, 0:256])
            nc.vector.tensor_copy(qk_sb[:, 0, 0:512], pQ[:])

            # (0,0): k256(j) covers b0 K tokens 256j:256j+256 (needed by
            # kp(j-1)); v_t needed by the pop at kp(t//2+3), so emitted by
            # kp(t//2+2); Q(0,1) by next block. kp0/kp1 extras run AFTER the
            # s_exps so the first exps start as early as possible.
            cy = emit_s2_pair(0, 0, post={
                0: [U(emit_k256, 1), U(emit_k256, 2)],
                1: [U(emit_k256, 3), U(emit_k256, 4), U(emit_v, 0)],
            }, extras={
                2: [U(emit_v, 1), U(emit_v, 2), U(emit_v, 3)],
                3: [U(emit_k256, 5), U(emit_v, 4), U(emit_v, 5)],
                4: [U(emit_k256, 6), U(emit_v, 6), U(emit_v, 7)],
                5: [U(emit_k256, 7), U(emit_v, 8), U(emit_v, 9)],
                6: [U(emit_qk, 0, 1), U(emit_v, 10), U(emit_v, 11)],
                7: [U(emit_v, 12), U(emit_v, 13)],
            })
            qk14 = emit_qk_split(1, 4)
            qk02 = emit_qk_split(0, 2)
            qk03 = emit_qk_split(0, 3)
            cy = emit_s2_pair(0, 1, carry=cy, extras={
                0: [U(emit_v, 14), U(emit_v, 15)],
                2: [U(emit_v, 16), U(emit_v, 17)],
                3: [qk02[0]],
                4: [qk02[1], U(emit_v, 18)],
                5: [qk14[0]],
                6: [qk14[1], U(emit_v, 19)],
                7: [qk03[0], U(emit_v, 20)],
            })
            qk15 = emit_qk_split(1, 5)
            qk04 = emit_qk_split(0, 4)
            qk05 = emit_qk_split(0, 5)
            cy = emit_s2_pair(0, 2, carry=cy, extras={
                2: [qk03[1], U(emit_v, 21)],
                3: [qk15[0]],
                4: [qk15[1], U(emit_v, 22)],
                5: [qk04[0]],
                6: [qk04[1], U(emit_v, 23)],
                7: [qk05[0], U(emit_v, 24)],
            })
            qk16 = emit_qk_split(1, 6)
            qk06 = emit_qk_split(0, 6)
            cy = emit_s2_pair(0, 3, carry=cy, extras={
                2: [qk05[1], U(emit_v, 25)],
                3: [qk16[0]],
                4: [qk16[1], U(emit_v, 26)],
                5: [qk06[0]],
                6: [qk06[1], U(emit_proj, 0)],
                7: [U(emit_proj, 1), U(emit_v, 27)],
            })
            qk17 = emit_qk_split(1, 7)
            qk07 = emit_qk_split(0, 7)
            cy = emit_s2_pair(1, 0, carry=cy, extras={
                2: [U(emit_v, 28), U(emit_v, 29)],
                3: [qk17[0]],
                4: [qk17[1], U(emit_v, 30)],
                5: [qk07[0]],
                6: [qk07[1], U(emit_v, 31)],
                7: [U(emit_proj, 2), U(emit_proj, 3)],
            })
            cy = emit_s2_pair(1, 1, carry=cy, extras={
                2: [U(emit_proj, 4)],
                3: [U(emit_proj, 5), U(emit_proj, 6)],
                4: [U(emit_proj, 7), U(emit_proj, 8)],
                5: [U(emit_proj, 9), U(emit_proj, 10)],
                6: [U(emit_proj, 11), U(emit_proj, 12)],
                7: [U(emit_proj, 13)],
            })
            cy = emit_s2_pair(1, 2, carry=cy, extras={
                2: [U(emit_proj, 14)],
                3: [U(emit_proj, 15), U(emit_proj, 16)],
                4: [U(emit_proj, 17), U(emit_proj, 18)],
                5: [U(emit_proj, 19), U(emit_proj, 20)],
                6: [U(emit_proj, 21), U(emit_proj, 22)],
                7: [U(emit_proj, 23)],
            })
            emit_s2_pair(1, 3, carry=cy, last=True, extras={
                2: [U(emit_proj, 24)],
                3: [U(emit_proj, 25)],
                4: [U(emit_proj, 26)],
                5: [U(emit_proj, 27)],
            })
            # tail: last 4 projections with evacuation split across scalar
            # (idle after the exp stream) and vector; out-DMAs spread over
            # queues
            emit_proj(28, evac="scalar", dma_eng=nc.scalar)
            emit_proj(29, evac=None, dma_eng=nc.sync)
            emit_proj(30, evac="scalar", dma_eng=nc.gpsimd)
            emit_proj(31, evac=None, dma_eng=nc.sync)
    nc.compile()
    return nc


def make_in_maps(x, w_qkv, w_proj):
    bf = ml_dtypes.bfloat16
    x2 = x.reshape(T, C)
    xT_np = np.ascontiguousarray(x2.T).astype(bf)
    in_maps = []
    for c in range(NCORES):
        s = c * 128
        wq = w_qkv[:, s:s + 128] * SCALE
        wk = w_qkv[:, C + s:C + s + 128]
        wqk_np = np.ascontiguousarray(
            np.concatenate([wq, wk], axis=1)).astype(bf)
        wv_np = np.ascontiguousarray(
            w_qkv[:, 2 * C + s:2 * C + s + 128]).astype(bf)
        wproj_np = np.ascontiguousarray(w_proj[s:s + 128, :]).astype(bf)
        in_maps.append({"xT": xT_np, "wqk": wqk_np, "wv": wv_np,
                        "wproj": wproj_np})
    return in_maps


def kernel(x, w_qkv, w_proj, b_proj):
    x = np.asarray(x, dtype=np.float32)
    w_qkv = np.asarray(w_qkv, dtype=np.float32)
    w_proj = np.asarray(w_proj, dtype=np.float32)
    b_proj = np.asarray(b_proj, dtype=np.float32)

    if "nc" not in _NC_CACHE:
        _NC_CACHE["nc"] = build()
    nc = _NC_CACHE["nc"]

    in_maps = make_in_maps(x, w_qkv, w_proj)
    res = run_bass_kernel_spmd(nc, in_maps, list(range(NCORES)))
    acc = np.zeros((T, C), dtype=np.float32)
    for r in res.results:
        acc += np.asarray(r["out"], dtype=np.float32)
    acc += b_proj[None, :]
    return acc.reshape(B, N, C)


# revision 12
# speedup vs baseline: 1.1890x; 1.0135x over previous
"""Multi-head attention forward (B=2, N=2048, C=1024, H=16) on 8 TRN2 NeuronCores.

Tensor-parallel over heads: core c owns heads {2c, 2c+1}. Each core computes
QKV projection for its heads, full attention for its 4 (batch, head)
instances, and a partial output projection against its 128 rows of w_proj.
The host sums the 8 partial projections and adds the bias (row-parallel TP;
the all-reduce is the host-side unshard).

Per-core layouts (all matmul inputs bf16, PSUM accumulation f32):
  xT    [1024, 4096]  x^T, channel-major (replicated)
  wqk   [1024, 256]   [Wq_h0|Wq_h1|Wk_h0|Wk_h1] columns, Wq pre-scaled by D^-0.5
  wv    [1024, 128]   [Wv_h0|Wv_h1]
  wproj [128, 1024]   rows 128c:128c+128 of w_proj
  out   [4096, 1024]  bf16 partial projection output

Attention processes BOTH heads per (batch, q-block): the two S^T matmuls
(K=64 contraction each) are issued back-to-back with tile_position row
tiles so they run CONCURRENTLY in disjoint halves of the PE array. Per
k-tile, one [128, 1024] PSUM tile holds [S^T_h0 | S^T_h1] and a single ACT
exp covers both heads.

V is computed directly in token-major layout (stationary = x^T chunks,
moving = wv, N=128) -- no PE transposes. Slot layouts:
  slot0 (h0): [V_h0 (64) | 1 | zeros(63)]   -> po0 = [O_h0 0:64; l_h0 @64]
  slot1 (h1): [1 | zeros(63) | V_h1 (64)]   -> po1 = [l_h1 @0; O_h1 64:128]
so h1's normalized output writes otp partitions 64:128 DIRECTLY (the DVE
mul reads the broadcast tile at partitions 0:64; cross-offset in1 reads are
supported) -- no SBUF repack DMA. l rides the PV matmul via the ones column.

Startup: critical DMAs (wqk + x^T tokens 0:512) spread over all four
queues; ~40 dummy ident matmuls warm the PE HAM clock-gate during the DMA
wait; a short prefix (K tokens 0:256 at N=256, Q tokens 0:512) gets the
first S-pair and exp stream going ~12us earlier than a full-block prefix.

Scheduling: stage-1 and projection PE work is woven into the attention
loops' ACT-wait slots. Exp->O matmuls are skewed 3 kp iterations; each
q-block's last 6 O-accumulations drain 3-per-kp into the next block's
kp0/kp1 with the normalize chain issued at kp1 so the po PSUM banks free
before the block's own O pops begin at kp3.
"""

import numpy as np
import ml_dtypes

import concourse.bass as bass
import concourse.tile as tile
from concourse import bacc, mybir
from concourse.bass_utils import run_bass_kernel_spmd
from concourse.masks import make_identity

B, N, C = 2, 2048, 1024
H = 16
D = C // H          # 64
SCALE = D ** -0.5
NCORES = 8
T = B * N           # 4096 tokens
KT = C // 128       # 8 k-tiles over the C contraction
NK = N // 128       # 16 key tiles per sequence
QB = 512            # q block width
BF = mybir.dt.bfloat16
F32 = mybir.dt.float32

_NC_CACHE = {}


def build():
    nc = bacc.Bacc("TRN2", target_bir_lowering=False, debug=False,
                   num_devices=NCORES)
    xT = nc.dram_tensor("xT", [C, T], BF, kind="ExternalInput").ap()
    wqk = nc.dram_tensor("wqk", [C, 256], BF, kind="ExternalInput").ap()
    wv = nc.dram_tensor("wv", [C, 128], BF, kind="ExternalInput").ap()
    wproj = nc.dram_tensor("wproj", [128, C], BF, kind="ExternalInput").ap()
    out = nc.dram_tensor("out", [T, C], BF, kind="ExternalOutput").ap()

    with tile.TileContext(nc) as tc:
        with tc.tile_pool(name="const", bufs=1) as const, \
             tc.tile_pool(name="work", bufs=2) as work, \
             tc.tile_pool(name="ps", bufs=2, space="PSUM") as ps:

            xt_sb = const.tile([128, KT, T], BF, tag="xt")
            wqk_sb = const.tile([128, KT, 256], BF, tag="wqk")
            wv_sb = const.tile([128, KT, 128], BF, tag="wv")
            wproj_sb = const.tile([128, C], BF, tag="wproj")
            qk_sb = const.tile([128, 2, T], BF, tag="qk")   # [qchan|kchan, token]
            # v slots per 128-token tile: [h0: V|1|0pad, h1: 0pad|1|V]
            v_sb = const.tile([128, T // 128, 2, 128], BF, tag="v")
            otp_sb = const.tile([128, T], BF, tag="otp")    # normalized O^T packed
            ident = const.tile([128, 128], BF, tag="ident")

            # ---- critical DMAs: wqk (8 chunks) + x^T tokens 0:512 (8
            # chunks) spread as the FIRST 4 descriptors on each of the four
            # queues. Everything later goes on sync/gpsimd.
            def xt_dma(eng, nt, kt):
                eng.dma_start(
                    out=xt_sb[:, kt, nt * 512:(nt + 1) * 512],
                    in_=xT[kt * 128:(kt + 1) * 128,
                           nt * 512:(nt + 1) * 512])

            for kt in range(KT):
                (nc.sync if kt % 2 == 0 else nc.scalar).dma_start(
                    out=wqk_sb[:, kt, :], in_=wqk[kt * 128:(kt + 1) * 128, :])
                if kt % 2 == 0:
                    xt_dma(nc.gpsimd, 0, kt)
                else:
                    xt_dma(nc.sync if kt % 4 == 1 else nc.gpsimd, 0, kt)

            # ---- v-slot constants; then HAM-prewarm dummy matmuls (no DMA
            # deps) so the PE clock-gate opens during the DMA wait
            nc.vector.memset(ident[:], 1.0)
            nc.vector.memset(v_sb[:, :, 0, 64:65], 1.0)
            nc.vector.memset(v_sb[:, :, 1, 0:1], 1.0)
            pwarm = ps.tile([128, 512], F32, tag="mm")
            for i in range(40):
                nc.tensor.matmul(pwarm[:, 0:128], ident[:], ident[:],
                                 start=True, stop=True)

            # ---- remaining input DMAs on sync/gpsimd only
            for kt in range(KT):
                xt_dma(nc.sync if kt % 2 == 0 else nc.gpsimd, 1, kt)
                nc.gpsimd.dma_start(out=wv_sb[:, kt, :],
                                    in_=wv[kt * 128:(kt + 1) * 128, :])
            nc.gpsimd.dma_start(out=wproj_sb[:], in_=wproj[:, :])
            for nt in range(2, T // 512):
                for kt in range(KT):
                    xt_dma(nc.sync if kt % 2 == 0 else nc.gpsimd, nt, kt)

            # -- work units -------------------------------------------------
            def emit_qk(mt, nt):
                pmm = ps.tile([128, 512], F32, tag="mm")
                for kt in range(KT):
                    nc.tensor.matmul(
                        pmm[:],
                        wqk_sb[:, kt, mt * 128:(mt + 1) * 128],
                        xt_sb[:, kt, nt * 512:(nt + 1) * 512],
                        start=(kt == 0), stop=(kt == KT - 1))
                nc.vector.tensor_copy(
                    qk_sb[:, mt, nt * 512:(nt + 1) * 512], pmm[:])

            def emit_qk_split(mt, nt):
                # 8-chunk GEMM split into two 4-chunk halves emitted in
                # consecutive extras slots (accumulation group + psum tile
                # stay open across the pair)
                st = {}

                def a():
                    pmm = ps.tile([128, 512], F32, tag="mm", name="pmm")
                    st["p"] = pmm
                    for kt in range(KT // 2):
                        nc.tensor.matmul(
                            pmm[:],
                            wqk_sb[:, kt, mt * 128:(mt + 1) * 128],
                            xt_sb[:, kt, nt * 512:(nt + 1) * 512],
                            start=(kt == 0), stop=False)

                def b():
                    pmm = st["p"]
                    for kt in range(KT // 2, KT):
                        nc.tensor.matmul(
                            pmm[:],
                            wqk_sb[:, kt, mt * 128:(mt + 1) * 128],
                            xt_sb[:, kt, nt * 512:(nt + 1) * 512],
                            start=False, stop=(kt == KT - 1))
                    nc.vector.tensor_copy(
                        qk_sb[:, mt, nt * 512:(nt + 1) * 512], pmm[:])

                return a, b

            def emit_k256(j):
                # K chunk for tokens j*256:(j+1)*256 (k-tiles 2j, 2j+1);
                # one N=256 GEMM + one CAST, fine-grained for early blocks
                pmm = ps.tile([128, 512], F32, tag="mm")
                for kt in range(KT):
                    nc.tensor.matmul(
                        pmm[:, 0:256],
                        wqk_sb[:, kt, 128:256],
                        xt_sb[:, kt, j * 256:(j + 1) * 256],
                        start=(kt == 0), stop=(kt == KT - 1))
                nc.vector.tensor_copy(
                    qk_sb[:, 1, j * 256:(j + 1) * 256], pmm[:, 0:256])

            def emit_v(t):
                # token-major V for 128-token tile t, BOTH heads:
                # out[tok, vchan] = sum_k xT[k, tok-tile]^T @ wv[k, :]
                pv = ps.tile([128, 512], F32, tag="mm")
                for kt in range(KT):
                    nc.tensor.matmul(
                        pv[:, 0:128],
                        xt_sb[:, kt, t * 128:(t + 1) * 128],
                        wv_sb[:, kt, :],
                        start=(kt == 0), stop=(kt == KT - 1))
                nc.vector.tensor_copy(v_sb[:, t, 0, 0:64], pv[:, 0:64])
                nc.vector.tensor_copy(v_sb[:, t, 1, 64:128], pv[:, 64:128])

            def emit_proj(g, evac=None, dma_eng=None):
                # out_tile = O^T_packed.T @ wproj (K=128, both heads)
                ob = work.tile([128, C], BF, tag="outstage", bufs=4)
                for ntile in range(2):
                    pmm = ps.tile([128, 512], F32, tag="mm")
                    nc.tensor.matmul(
                        pmm[:],
                        otp_sb[:, g * 128:(g + 1) * 128],
                        wproj_sb[:, ntile * 512:(ntile + 1) * 512],
                        start=True, stop=True)
                    if evac == "scalar":
                        nc.scalar.copy(
                            ob[:, ntile * 512:(ntile + 1) * 512], pmm[:])
                    elif evac == "mixed" and ntile == 1:
                        nc.scalar.copy(ob[:, 512:1024], pmm[:])
                    else:
                        nc.vector.tensor_copy(
                            ob[:, ntile * 512:(ntile + 1) * 512], pmm[:])
                (dma_eng or nc.sync).dma_start(
                    out=out[g * 128:(g + 1) * 128, :], in_=ob[:])

            # One attention q-block, BOTH heads.
            def emit_s2_pair(b, qb, extras=None, post=None, carry=None,
                             last=False):
                q0 = b * N + qb * QB
                po0 = ps.tile([128, 512], F32, tag="o0", bufs=1)
                po1 = ps.tile([128, 512], F32, tag="o1", bufs=1)
                from collections import deque as _dq
                pending = _dq()

                def pop_o():
                    es, kt = pending.popleft()
                    vt = b * NK + kt
                    nc.tensor.matmul(
                        po0[:], v_sb[:, vt, 0, :], es[:, 0:512],
                        start=(kt == 0), stop=(kt == NK - 1))
                    nc.tensor.matmul(
                        po1[:], v_sb[:, vt, 1, :], es[:, 512:1024],
                        start=(kt == 0), stop=(kt == NK - 1))

                def normalize(heads=(0, 1)):
                    # h0: l at po0 row 64, O at rows 0:64 -> otp[0:64]
                    # h1: l at po1 row 0, O at rows 64:128 -> otp[64:128]
                    for h in heads:
                        po = po0 if h == 0 else po1
                        lrow = work.tile([1, 512], F32, tag="lrow")
                        nc.vector.tensor_copy(
                            lrow[:], po[64:65, :] if h == 0 else po[0:1, :])
                        nc.vector.reciprocal_approx_fast(lrow[:], lrow[:])
                        lb = work.tile([64, 512], F32, tag="lb")
                        nc.gpsimd.partition_broadcast(lb[:], lrow[:])
                        if h == 0:
                            nc.vector.tensor_mul(
                                otp_sb[0:64, q0:q0 + QB], po[0:64, :], lb[:])
                        else:
                            nc.vector.tensor_mul(
                                otp_sb[64:128, q0:q0 + QB], po[64:128, :],
                                lb[:])

                def s_exp(kp, j):
                    kt = kp * 2 + j
                    k0 = b * N + kt * 128
                    pst = ps.tile([128, 1024], F32, tag="s")
                    nc.tensor.matmul(
                        pst[:, 0:512],
                        qk_sb[0:64, 1, k0:k0 + 128],
                        qk_sb[0:64, 0, q0:q0 + QB],
                        start=True, stop=True)
                    nc.tensor.matmul(
                        pst[:, 512:1024],
                        qk_sb[64:128, 1, k0:k0 + 128],
                        qk_sb[64:128, 0, q0:q0 + QB],
                        start=True, stop=True)
                    es = work.tile([128, 1024], BF, tag="es", bufs=13)
                    nc.scalar.activation(
                        es[:], pst[:], mybir.ActivationFunctionType.Exp)
                    pending.append((es, kt))

                for kp in range(NK // 2):
                    if carry is not None:
                        carry[0]()
                        carry[0]()
                        if carry[1]():
                            carry = None
                    else:
                        if len(pending) >= 6:
                            pop_o()
                            pop_o()
                    if extras:
                        for u in extras.get(kp, ()):
                            u()
                    s_exp(kp, 0)
                    s_exp(kp, 1)
                    if post:
                        for u in post.get(kp, ()):
                            u()
                if last:
                    # drain head-by-head: h0's normalize chain overlaps h1's
                    # remaining O matmuls on the PE; normalize in 256-col
                    # halves so the first tail projections start early
                    rem = list(pending)
                    pending.clear()
                    for es, kt in rem:
                        nc.tensor.matmul(
                            po0[:], v_sb[:, b * NK + kt, 0, :], es[:, 0:512],
                            start=(kt == 0), stop=(kt == NK - 1))
                    normalize(heads=(0,))
                    for es, kt in rem:
                        nc.tensor.matmul(
                            po1[:], v_sb[:, b * NK + kt, 1, :],
                            es[:, 512:1024],
                            start=(kt == 0), stop=(kt == NK - 1))
                    normalize(heads=(1,))
                    return None

                def carry_pop():
                    if pending:
                        pop_o()

                def carry_fin():
                    if pending:
                        return False
                    normalize()
                    return True

                return (carry_pop, carry_fin)

            # -- schedule ---------------------------------------------------
            def U(f, *a):
                return lambda: f(*a)

            # minimal prefix for (b0, qb0): K tokens 0:256 (N=256) and the
            # full Q block 0:512, K/Q interleaved per k-chunk
            pK = ps.tile([128, 512], F32, tag="mm")
            pQ = ps.tile([128, 512], F32, tag="mm")
            for kt in range(KT):
                nc.tensor.matmul(pK[:, 0:256], wqk_sb[:, kt, 128:256],
                                 xt_sb[:, kt, 0:256],
                                 start=(kt == 0), stop=(kt == KT - 1))
                nc.tensor.matmul(pQ[:], wqk_sb[:, kt, 0:128],
                                 xt_sb[:, kt, 0:512],
                                 start=(kt == 0), stop=(kt == KT - 1))
            nc.vector.tensor_copy(qk_sb[:, 1, 0:256], pK[:, 0:256])
            nc.vector.tensor_copy(qk_sb[:, 0, 0:512], pQ[:])
            # zero-pads of the v slots: issued after the prefix CASTs so the
            # first exps aren't queued behind these bulk memsets on DVE;
            # needed only by the first O pop at (0,0) kp3
            nc.vector.memset(v_sb[:, :, 0, 65:128], 0.0)
            nc.vector.memset(v_sb[:, :, 1, 1:64], 0.0)

            # (0,0): k256(j) covers b0 K tokens 256j:256j+256 (needed by
            # kp(j-1)); v_t needed by the pop at kp(t//2+3), so emitted by
            # kp(t//2+2); Q(0,1) by next block. kp0/kp1 extras run AFTER the
            # s_exps so the first exps start as early as possible.
            cy = emit_s2_pair(0, 0, post={
                0: [U(emit_k256, 1), U(emit_k256, 2)],
                1: [U(emit_k256, 3), U(emit_k256, 4), U(emit_v, 0)],
            }, extras={
                2: [U(emit_v, 1), U(emit_v, 2), U(emit_v, 3)],
                3: [U(emit_k256, 5), U(emit_v, 4), U(emit_v, 5)],
                4: [U(emit_k256, 6), U(emit_v, 6), U(emit_v, 7)],
                5: [U(emit_k256, 7), U(emit_v, 8), U(emit_v, 9)],
                6: [U(emit_qk, 0, 1), U(emit_v, 10), U(emit_v, 11)],
                7: [U(emit_v, 12), U(emit_v, 13)],
            })
            qk14 = emit_qk_split(1, 4)
            qk02 = emit_qk_split(0, 2)
            qk03 = emit_qk_split(0, 3)
            cy = emit_s2_pair(0, 1, carry=cy, extras={
                0: [U(emit_v, 14), U(emit_v, 15)],
                2: [U(emit_v, 16), U(emit_v, 17)],
                3: [qk02[0]],
                4: [qk02[1], U(emit_v, 18)],
                5: [qk14[0]],
                6: [qk14[1], U(emit_v, 19)],
                7: [qk03[0], U(emit_v, 20)],
            })
            qk15 = emit_qk_split(1, 5)
            qk04 = emit_qk_split(0, 4)
            qk05 = emit_qk_split(0, 5)
            cy = emit_s2_pair(0, 2, carry=cy, extras={
                2: [qk03[1], U(emit_v, 21)],
                3: [qk15[0]],
                4: [qk15[1], U(emit_v, 22)],
                5: [qk04[0]],
                6: [qk04[1], U(emit_v, 23)],
                7: [qk05[0], U(emit_v, 24)],
            })
            qk16 = emit_qk_split(1, 6)
            qk06 = emit_qk_split(0, 6)
            cy = emit_s2_pair(0, 3, carry=cy, extras={
                2: [qk05[1], U(emit_v, 25)],
                3: [qk16[0]],
                4: [qk16[1], U(emit_v, 26)],
                5: [qk06[0]],
                6: [qk06[1], U(emit_proj, 0)],
                7: [U(emit_proj, 1), U(emit_v, 27)],
            })
            qk17 = emit_qk_split(1, 7)
            qk07 = emit_qk_split(0, 7)
            cy = emit_s2_pair(1, 0, carry=cy, extras={
                2: [U(emit_v, 28), U(emit_v, 29)],
                3: [qk17[0]],
                4: [qk17[1], U(emit_v, 30)],
                5: [qk07[0]],
                6: [qk07[1], U(emit_v, 31)],
                7: [U(emit_proj, 2, "mixed"), U(emit_proj, 3, "mixed")],
            })
            cy = emit_s2_pair(1, 1, carry=cy, extras={
                2: [U(emit_proj, 4)],
                3: [U(emit_proj, 5), U(emit_proj, 6)],
                4: [U(emit_proj, 7), U(emit_proj, 8)],
                5: [U(emit_proj, 9), U(emit_proj, 10)],
                6: [U(emit_proj, 11), U(emit_proj, 12)],
                7: [U(emit_proj, 13, "mixed")],
            })
            cy = emit_s2_pair(1, 2, carry=cy, extras={
                2: [U(emit_proj, 14)],
                3: [U(emit_proj, 15), U(emit_proj, 16)],
                4: [U(emit_proj, 17), U(emit_proj, 18)],
                5: [U(emit_proj, 19), U(emit_proj, 20)],
                6: [U(emit_proj, 21), U(emit_proj, 22)],
                7: [U(emit_proj, 23, "mixed")],
            })
            emit_s2_pair(1, 3, carry=cy, last=True, extras={
                2: [U(emit_proj, 24)],
                3: [U(emit_proj, 25)],
                4: [U(emit_proj, 26)],
                5: [U(emit_proj, 27)],
            })
            # tail: last 4 projections with evacuation split across scalar
            # (idle after the exp stream) and vector; out-DMAs spread over
            # queues
            emit_proj(28, evac="scalar", dma_eng=nc.scalar)
            emit_proj(29, evac=None, dma_eng=nc.sync)
            emit_proj(30, evac="scalar", dma_eng=nc.gpsimd)
            emit_proj(31, evac=None, dma_eng=nc.sync)
    nc.compile()
    return nc


def make_in_maps(x, w_qkv, w_proj):
    bf = ml_dtypes.bfloat16
    x2 = x.reshape(T, C)
    xT_np = np.ascontiguousarray(x2.T).astype(bf)
    in_maps = []
    for c in range(NCORES):
        s = c * 128
        wq = w_qkv[:, s:s + 128] * SCALE
        wk = w_qkv[:, C + s:C + s + 128]
        wqk_np = np.ascontiguousarray(
            np.concatenate([wq, wk], axis=1)).astype(bf)
        wv_np = np.ascontiguousarray(
            w_qkv[:, 2 * C + s:2 * C + s + 128]).astype(bf)
        wproj_np = np.ascontiguousarray(w_proj[s:s + 128, :]).astype(bf)
        in_maps.append({"xT": xT_np, "wqk": wqk_np, "wv": wv_np,
                        "wproj": wproj_np})
    return in_maps


def kernel(x, w_qkv, w_proj, b_proj):
    x = np.asarray(x, dtype=np.float32)
    w_qkv = np.asarray(w_qkv, dtype=np.float32)
    w_proj = np.asarray(w_proj, dtype=np.float32)
    b_proj = np.asarray(b_proj, dtype=np.float32)

    if "nc" not in _NC_CACHE:
        _NC_CACHE["nc"] = build()
    nc = _NC_CACHE["nc"]

    in_maps = make_in_maps(x, w_qkv, w_proj)
    res = run_bass_kernel_spmd(nc, in_maps, list(range(NCORES)))
    acc = np.zeros((T, C), dtype=np.float32)
    for r in res.results:
        acc += np.asarray(r["out"], dtype=np.float32)
    acc += b_proj[None, :]
    return acc.reshape(B, N, C)
